# revision 1
# baseline (speedup 1.0000x reference)
"""Trainium2 Bass kernel for the tiny NeRF MLP (nn_NeRFtinymodel).

Network (per point):
    h1 = relu(emb @ W_in + b_in)            # 32 -> 64
    h2 = relu(h1 @ W0 + b0)                 # 64 -> 64
    x3 = h2 @ Wd + bd                       # 64 -> 16 (no relu)
    dense = x3[:, 0]
    h3 = relu([x3[:,1:], enc_dir] @ Wc + bc)  # (15+39) -> 64
    h4 = relu(h3 @ W1a + b1a)
    h5 = relu(h4 @ W1b + b1b)
    color = h5 @ Wo + bo
    out = [color, dense]

Wd/Wc are algebraically fused on the host (no relu between them):
    h3 = relu(h2 @ (Wd[:,1:]@Wc[:15]) + enc_dir @ Wc[15:] + bc')
    dense = h2 @ Wd[:,0] + bd[0]

Device structure ("variant C2"): activations kept transposed
[features, points], 512 points per tile, tiles processed in groups of
4 with a 4-tile software-pipeline skew.  The six matmul stages fuse
pairwise into three K=128/M=128 block-diagonal float32r matmuls per
tile (float32r = 1 column/cycle at near-fp32 precision; its ISA only
allows PSUM output base 0, which this layout satisfies):

    P1(t): rhs=[h3(t-4) | outs | X(t)] -> [h4pre(t-4) | h1pre(t)]
    P2(t): rhs=[h4(t-4) | h1(t)]       -> [h2pre(t)   | h5pre(t-4)]
    P3(t): rhs=[h2(t)   | h5(t-4)]     -> [h3pre(t) | den±(t) | col±(t-4)]
           + bf16 rider accumulating enc_dir @ Wc2 onto h3pre

color/dense use relu(x)-relu(-x) so the shared bias+relu eltwise
(always full 128 partitions) covers them; the host subtracts (exact).
All four tiles of a group share one 4-bank PSUM tile for P3 and one
wide eltwise; X / enc_dir / outputs move in 3 large DMAs per group
(HWDGE costs ~625ns per DMA instruction, so instruction count rules).

Sharding: pure data parallel over 8 cores on the points axis.
"""

import numpy as np

import concourse.bacc as bacc
import concourse.mybir as mybir
from concourse.tile import TileContext
from concourse.bass_utils import run_bass_kernel_spmd

N_CORES = 8
N_TOTAL = 1048576
NPC = N_TOTAL // N_CORES  # 131072 points per core
F = 512                   # points per tile (one PSUM bank)
G = 4                     # tiles per group
SKEW = 2                  # pipeline skew in groups
B1_BUFS = 3
B2_BUFS = 3
B3_BUFS = 1

f32 = mybir.dt.float32
f32r = mybir.dt.float32r
bf16 = mybir.dt.bfloat16
RELU = mybir.ActivationFunctionType.Relu
ADD = mybir.AluOpType.add
MAX = mybir.AluOpType.max


def build_program(npc=NPC, reps=1):
    assert npc % (G * F) == 0
    n_groups = npc // (G * F)

    nc = bacc.Bacc("TRN2", target_bir_lowering=False, debug=False,
                   num_devices=N_CORES)
    xT = nc.dram_tensor("xT", [32, npc], f32r, kind="ExternalInput")
    dT = nc.dram_tensor("dT", [128, npc // 2], bf16, kind="ExternalInput")
    wb = nc.dram_tensor("wb", [128, 384], f32r, kind="ExternalInput")
    we = nc.dram_tensor("we", [128, 256], bf16, kind="ExternalInput")
    bb = nc.dram_tensor("bb", [128, 3], f32, kind="ExternalInput")
    oT = nc.dram_tensor("oT", [8, npc], f32r, kind="ExternalOutput")

    with TileContext(nc) as tc:
        with (
            tc.tile_pool(name="wpool", bufs=1) as wpool,
            tc.tile_pool(name="io", bufs=2) as io,
            tc.tile_pool(name="act", bufs=3) as act,
            tc.tile_pool(name="ps", bufs=1, space="PSUM") as ps,
        ):
            wsb = wpool.tile([128, 384], f32r, name="wsb")
            esb = wpool.tile([128, 256], bf16, name="esb")
            bsb = wpool.tile([128, 3], f32, name="bsb")
            nc.sync.dma_start(out=wsb[:], in_=wb[:, :])
            nc.sync.dma_start(out=esb[:], in_=we[:, :])
            nc.sync.dma_start(out=bsb[:], in_=bb[:, :])
            L1 = wsb[:, 0:128]
            L2 = wsb[:, 128:256]
            L3 = wsb[:, 256:384]

            def bias_relu(on_act, dst, src, bias_col):
                b_ap = bsb[:, bias_col:bias_col + 1]
                if on_act:
                    nc.scalar.activation(dst, src, RELU, bias=b_ap, scale=1.0)
                else:
                    nc.vector.tensor_scalar(
                        out=dst, in0=src, scalar1=b_ap, scalar2=0.0,
                        op0=ADD, op1=MAX)

            for rep in range(reps):
                # two zeroed stand-ins for t3w(-2), t3w(-1); pipeline skew is
                # 2 groups (8 tiles) so E3 of group g-1 overlaps group g.
                t3q = []
                for i in range(SKEW):
                    t3p = act.tile([128, G * F], f32r, name="t3p", tag="t3",
                                   bufs=SKEW + 2)
                    nc.vector.memset(t3p[:].bitcast(f32), 0.0)
                    if i * G * F < npc:
                        nc.sync.dma_start(
                            out=t3p[96:128, :],
                            in_=xT[:, i * G * F:(i + 1) * G * F])
                    t3q.append(t3p)
                for g in range(n_groups + SKEW):
                    live = g < n_groups
                    t3p = t3q[g]
                    if live:
                        ds = io.tile([128, (G // 2) * F], bf16, name="ds",
                                     tag="ds")
                        nc.sync.dma_start(
                            out=ds[:],
                            in_=dT[:, (G // 2) * g * F:
                                   (G // 2) * (g + 1) * F])
                    b3s_list = [
                        ps.tile([128, 2 * F], f32, name=f"b3_{p}", tag="b3p",
                                bufs=B3_BUFS)
                        for p in range(G // 2)
                    ]
                    t3w = act.tile([128, G * F], f32r, name="t3w", tag="t3",
                                    bufs=SKEW + 2)
                    for k in range(G):
                        rhs1 = t3p[:, k * F:(k + 1) * F]
                        b1 = ps.tile([128, F], f32, name="b1", tag="b1",
                                     bufs=B1_BUFS)
                        t1 = act.tile([128, F], f32r, name="t1", tag="t1")
                        nc.tensor.matmul(b1[:, :], L1, rhs1,
                                         start=True, stop=True)
                        bias_relu(k in (0, 2), t1[:], b1[:], 0)
                        b2 = ps.tile([128, F], f32, name="b2", tag="b2",
                                     bufs=B2_BUFS)
                        t2 = act.tile([128, F], f32r, name="t2", tag="t2")
                        nc.tensor.matmul(b2[:, :], L2, t1[:, :],
                                         start=True, stop=True)
                        bias_relu(k in (0, 2), t2[:], b2[:], 1)
                        b3h = b3s_list[k // 2]
                        b3s = b3h[:, (k % 2) * F:(k % 2 + 1) * F]
                        if live:
                            nc.tensor.matmul(b3s, L3, t2[:, :],
                                             start=True, stop=False)
                            er = esb[:, 128 * (k % 2):128 * (k % 2) + 128]
                            dslot = ds[:, (k // 2) * F:(k // 2 + 1) * F]
                            nc.tensor.matmul(b3s, er, dslot,
                                             start=False, stop=True)
                        else:
                            nc.tensor.matmul(b3s, L3, t2[:, :],
                                             start=True, stop=True)
                    for p in range(G // 2):
                        bias_relu(p % 2 == 0,
                                  t3w[:, 2 * p * F:2 * (p + 1) * F],
                                  b3s_list[p][:], 2)
                    if live:
                        nc.sync.dma_start(
                            out=oT[:, g * G * F:(g + 1) * G * F],
                            in_=t3w[64:72, :])
                    else:
                        c0 = (g - n_groups) * G * F
                        nc.sync.dma_start(out=oT[2:8, c0:c0 + G * F],
                                          in_=t3w[66:72, :])
                    if g + SKEW < n_groups:
                        nc.sync.dma_start(
                            out=t3w[96:128, :],
                            in_=xT[:, (g + SKEW) * G * F:
                                   (g + SKEW + 1) * G * F])
                    t3q.append(t3w)
    nc.compile()
    return nc


def _host_prep(inputs):
    W_in, b_in = inputs["W_in"], inputs["b_in"]
    W0, b0 = inputs["W0"], inputs["b0"]
    Wd, bd = inputs["Wd"], inputs["bd"]
    Wc, bc = inputs["Wc"], inputs["bc"]
    W1a, b1a = inputs["W1a"], inputs["b1a"]
    W1b, b1b = inputs["W1b"], inputs["b1b"]
    Wo, bo = inputs["Wo"], inputs["bo"]

    Wc1 = (Wd[:, 1:].astype(np.float64) @ Wc[:15].astype(np.float64))
    bcp = (bd[1:].astype(np.float64) @ Wc[:15].astype(np.float64)
           + bc.astype(np.float64)).astype(np.float32)

    wblob = np.zeros((128, 384), np.float32)
    # L1: rows 0:64 = W1a (h3->h4pre) -> cols 0:64 ;
    #     rows 96:128 = W_in (X->h1pre) -> cols 64:128
    wblob[0:64, 0:64] = W1a
    wblob[96:128, 64:128] = W_in
    # L2: rows 0:64 = W1b (h4->h5pre) -> cols 64:128 ;
    #     rows 64:128 = W0 (h1->h2pre) -> cols 0:64
    wblob[0:64, 128 + 64:128 + 128] = W1b
    wblob[64:128, 128:128 + 64] = W0
    # L3: rows 0:64 (h2): Wc1 -> cols 0:64, +-Wd0 -> cols 64:66
    #     rows 64:128 (h5): +-Wo -> cols 66:72
    wblob[0:64, 256:256 + 64] = Wc1.astype(np.float32)
    wblob[0:64, 256 + 64] = Wd[:, 0]
    wblob[0:64, 256 + 65] = -Wd[:, 0]
    wblob[64:128, 256 + 66:256 + 69] = Wo
    wblob[64:128, 256 + 69:256 + 72] = -Wo

    # enc rider lhsT (bf16): even tiles contract rows 0:39, odd 39:78
    eblob = np.zeros((128, 256), np.float32)
    eblob[0:39, 0:64] = Wc[15:54]
    eblob[39:78, 128:192] = Wc[15:54]

    bblob = np.zeros((128, 3), np.float32)
    bblob[0:64, 0] = b1a
    bblob[64:128, 0] = b_in
    bblob[0:64, 1] = b0
    bblob[64:128, 1] = b1b
    bblob[0:64, 2] = bcp
    bblob[64, 2] = bd[0]
    bblob[65, 2] = -bd[0]
    bblob[66:69, 2] = bo
    bblob[69:72, 2] = -bo

    np_bf = mybir.dt.np(bf16)
    emb = inputs["emb_points"]
    enc = inputs["enc_dir"]
    in_maps = []
    for cc in range(N_CORES):
        sl = slice(cc * NPC, (cc + 1) * NPC)
        encT = np.ascontiguousarray(enc[sl].T).astype(np_bf)  # [39, NPC]
        # pair-interleaved, zero-padded enc blob [128, NPC//2]
        dpad = np.zeros((128, NPC // 2), np_bf)
        e4 = encT.reshape(39, NPC // (2 * F), 2, F)
        dpad[0:39] = e4[:, :, 0, :].reshape(39, NPC // 2)
        dpad[39:78] = e4[:, :, 1, :].reshape(39, NPC // 2)
        in_maps.append({
            "xT": np.ascontiguousarray(emb[sl].T),
            "dT": dpad,
            "wb": wblob,
            "we": eblob.astype(np_bf),
            "bb": bblob,
        })
    return in_maps


_PROGRAM_CACHE = {}


def _get_program(npc=NPC, reps=1):
    key = (npc, reps)
    if key not in _PROGRAM_CACHE:
        _PROGRAM_CACHE[key] = build_program(npc, reps)
    return _PROGRAM_CACHE[key]


def kernel(**inputs) -> np.ndarray:
    nc = _get_program(NPC, 1)
    in_maps = _host_prep(inputs)
    res = run_bass_kernel_spmd(nc, in_maps, core_ids=list(range(N_CORES)))
    out = np.empty((N_TOTAL, 4), np.float32)
    for cc in range(N_CORES):
        o = res.results[cc]["oT"]          # [8, NPC]
        sl = slice(cc * NPC, (cc + 1) * NPC)
        out[sl, 3] = o[0] - o[1]           # dense
        # color of tile t is stored at tile slot t+8 (mod n_tiles)
        col = o[2:5] - o[5:8]              # [3, NPC]
        out[sl, 0:3] = np.roll(col, -SKEW * G * F, axis=1).T
    return out



# revision 35
# speedup vs baseline: 1.2004x; 1.2004x over previous
"""Trainium2 Bass kernel for the tiny NeRF MLP (nn_NeRFtinymodel).

Network (per point):
    h1 = relu(emb @ W_in + b_in)            # 32 -> 64
    h2 = relu(h1 @ W0 + b0)                 # 64 -> 64
    x3 = h2 @ Wd + bd                       # 64 -> 16 (no relu)
    dense = x3[:, 0]
    h3 = relu([x3[:,1:], enc_dir] @ Wc + bc)  # (15+39) -> 64
    h4 = relu(h3 @ W1a + b1a)
    h5 = relu(h4 @ W1b + b1b)
    color = h5 @ Wo + bo
    out = [color, dense]

Wd/Wc are algebraically fused on the host (no relu between them):
    h3 = relu(h2 @ (Wd[:,1:]@Wc[:15]) + enc_dir @ Wc[15:] + bc')
    dense = h2 @ Wd[:,0] + bd[0]

Device structure ("variant F"): activations transposed [features,
points], F=512 points per tile/chain.  Chain of tile t carries the
head of tile t and the tail of tile t-12 through three K=128/M=128
block-diagonal float32r matmuls plus a K=80 bf16 enc rider (39 enc
rows + a constant-1 row carrying the h3 bias, pair-interleaved so two
tiles share each rider rhs column block):

    P1(t): rhs=[h3(t-12) | X(t)]  -> [h4pre(t-12) | h1pre(t)]
    P2(t): rhs=[h4(t-12) | h1(t)] -> [h2pre(t)    | h5pre(t-12)]
    P3(t): rhs=[h2(t)    | h5(t-12)] -> [h3pre+bias(t) | den+-(t) | col+-(t-12)]
           + rider accumulating [enc(t);1] @ [Wc2;bc'] onto rows 0:64

Modulo software pipeline, one step per tile; PE stream at step s is
P1(s), P2(s-4), P3(s-6)+rider.  The cost model charges a DMA's full
per-partition-byte time (0.39 ns/B) to the ISSUING queue engine and
an eltwise op's column count to its engine, so the design splits work
four ways:  t1 as a [128,1024] PAIR op on DVE,  t2 per-tile on Act,
t3 per-tile alternating DVE/Act,  xT loads on SP,  ds/oT DMAs on the
otherwise-idle Pool queue (SWDGE).  PSUM: b1 pair (2 banks) x2 + b2
x2 + b3 x2 = 8 banks exactly.  The output is packed [128, npc/16] so
the store costs 500 ns instead of 3158 (host unpacks); den/color ride
relu(x)-relu(-x) with NO device bias; host subtracts and adds
bd[0]/bo (exact).

Sharding: pure data parallel over 8 cores on the points axis.
"""

import numpy as np

import concourse.bacc as bacc
import concourse.mybir as mybir
from concourse.tile import TileContext
from concourse.bass_utils import run_bass_kernel_spmd

N_CORES = 8
N_TOTAL = 1048576
NPC = N_TOTAL // N_CORES  # 131072 points per core
F = 512                   # points per tile (one PSUM bank)
G = 4                     # tiles per group (xT/oT DMA granularity)
SKEW = 3                  # cross-chain skew in groups (d = SKEW*G tiles)
B1_BUFS = 2               # [128, 2F] pairs
B2_BUFS = 2
B3_BUFS = 2
T1_BUFS = 2               # [128, 2F] pairs
T2_BUFS = 2
T3_BUFS = SKEW + 2
DS_BUFS = 4

f32 = mybir.dt.float32
f32r = mybir.dt.float32r
bf16 = mybir.dt.bfloat16
RELU = mybir.ActivationFunctionType.Relu
ADD = mybir.AluOpType.add
MAX = mybir.AluOpType.max


def build_program(npc=NPC, reps=1):
    assert npc % (G * F) == 0 and npc % 16 == 0
    n_groups = npc // (G * F)
    T = npc // F                 # live tiles/chains
    TE = T + SKEW * G            # incl. epilogue chains

    nc = bacc.Bacc("TRN2", target_bir_lowering=False, debug=False,
                   num_devices=N_CORES)
    xT = nc.dram_tensor("xT", [32, npc], f32r, kind="ExternalInput")
    dT = nc.dram_tensor("dT", [80, npc // 2], bf16, kind="ExternalInput")
    wb = nc.dram_tensor("wb", [128, 384], f32r, kind="ExternalInput")
    we = nc.dram_tensor("we", [80, 256], bf16, kind="ExternalInput")
    bb = nc.dram_tensor("bb", [128, 2], f32, kind="ExternalInput")
    # packed output: col block g holds the group-g [8, 2048] staging rows
    # in row-major 16-chunk order (host unpacks)
    oT = nc.dram_tensor("oT", [128, npc // 16], f32r, kind="ExternalOutput")

    with TileContext(nc) as tc:
        with (
            tc.tile_pool(name="wpool", bufs=1) as wpool,
            tc.tile_pool(name="io", bufs=2) as io,
            tc.tile_pool(name="act", bufs=3) as act,
            tc.tile_pool(name="ps", bufs=1, space="PSUM") as ps,
        ):
            wsb = wpool.tile([128, 384], f32r, name="wsb")
            esb = wpool.tile([80, 256], bf16, name="esb")
            bsb = wpool.tile([128, 2], f32, name="bsb")
            nc.sync.dma_start(out=wsb[:], in_=wb[:, :])
            nc.scalar.dma_start(out=esb[:], in_=we[:, :])
            nc.scalar.dma_start(out=bsb[:], in_=bb[:, :])
            L1 = wsb[:, 0:128]
            L2 = wsb[:, 128:256]
            L3 = wsb[:, 256:384]

            for rep in range(reps):
                t3tile = {}   # target-group -> [128, G*F] sbuf tile
                b1p, b2t, b3t = {}, {}, {}
                t1p, t2t, dst = {}, {}, {}
                t3_made = 0   # t3 tag instances created this rep

                # prime: zero stand-ins for chain-groups 0..SKEW-1.
                # Spread memsets/X loads over queues so the first chains
                # start as early as possible.
                x_eng = [nc.sync, nc.sync, nc.sync]
                m_eng = [nc.gpsimd, nc.vector, nc.gpsimd]
                for g in range(SKEW):
                    tp = act.tile([128, G * F], f32r, name="t3p", tag="t3",
                                  bufs=T3_BUFS)
                    t3_made += 1
                    m_eng[g].memset(tp[0:96, :].bitcast(f32), 0.0)
                    # load X (any region for g >= n_groups: rows just need
                    # to be finite/owned; cheaper than a memset)
                    gx = min(g, n_groups - 1)
                    x_eng[g].dma_start(
                        out=tp[96:128, :],
                        in_=xT[:, gx * G * F:(gx + 1) * G * F])
                    t3tile[g] = tp
                if n_groups > 0:
                    d0 = io.tile([80, (G // 2) * F], bf16, name="ds",
                                 tag="ds", bufs=DS_BUFS)
                    nc.gpsimd.dma_start(out=d0[:], in_=dT[:, 0:(G // 2) * F])
                    dst[0] = d0

                for s in range(TE + 8):
                    # ---- eltwises for results of previous steps ----
                    u = s - 2
                    if 0 <= u < TE and u % 2 == 0:
                        # t1 pair q=u//2 = relu(b1p + bias0)  [DVE]
                        q = u // 2
                        t1 = act.tile([128, 2 * F], f32r, name="t1",
                                      tag="t1", bufs=T1_BUFS)
                        nc.vector.tensor_scalar(
                            out=t1[:], in0=b1p.pop(q)[:],
                            scalar1=bsb[:, 0:1], scalar2=0.0,
                            op0=ADD, op1=MAX)
                        t1p[q] = t1
                    u = s - 5
                    if 0 <= u < TE:      # t2 = relu(b2 + bias1)  [Act]
                        t2 = act.tile([128, F], f32r, name="t2", tag="t2",
                                      bufs=T2_BUFS)
                        nc.scalar.activation(t2[:], b2t.pop(u)[:], RELU,
                                             bias=bsb[:, 1:2], scale=1.0)
                        t2t[u] = t2
                    u = s - 7
                    if 0 <= u < TE:
                        # t3(u): per-tile relu of rows 0:96 [DVE/Act alt]
                        gt = u // 4 + SKEW
                        if gt not in t3tile:
                            t3tile[gt] = act.tile(
                                [128, G * F], f32r, name="t3w",
                                tag="t3", bufs=T3_BUFS)
                            t3_made += 1
                            # real X, or (epilogue) any X region: rows
                            # 96:128 just need to be owned/finite for P1
                            gx = min(gt, n_groups - 1)
                            nc.sync.dma_start(
                                out=t3tile[gt][96:128, :],
                                in_=xT[:, gx * G * F:
                                       (gx + 1) * G * F])
                        c0 = (u % 4) * F
                        dstv = t3tile[gt][0:96, c0:c0 + F]
                        srcv = b3t.pop(u)[0:96, :]
                        if u % 2 == 1:
                            nc.vector.tensor_scalar(
                                out=dstv, in0=srcv, scalar1=0.0,
                                scalar2=None, op0=MAX)
                        else:
                            nc.scalar.activation(dstv, srcv, RELU,
                                                 bias=0.0, scale=1.0)

                    # ---- matmuls ----
                    if s < TE:           # P1(s) into b1 pair slot
                        t = s
                        q = t // 2
                        if t % 2 == 0:
                            b1p[q] = ps.tile([128, 2 * F], f32, name="b1",
                                             tag="b1", bufs=B1_BUFS)
                        rhs = t3tile[t // 4][:, (t % 4) * F:(t % 4 + 1) * F]
                        nc.tensor.matmul(
                            b1p[q][:, (t % 2) * F:(t % 2 + 1) * F],
                            L1, rhs, start=True, stop=True)
                    t = s - 4
                    if 0 <= t < TE:      # P2(t)
                        b2 = ps.tile([128, F], f32, name="b2", tag="b2",
                                     bufs=B2_BUFS)
                        nc.tensor.matmul(
                            b2[:, :], L2,
                            t1p[t // 2][:, (t % 2) * F:(t % 2 + 1) * F],
                            start=True, stop=True)
                        b2t[t] = b2
                        if t % 2 == 1:
                            del t1p[t // 2]
                    t = s - 6
                    if 0 <= t < TE:      # P3(t) (+ rider for live tiles)
                        b3 = ps.tile([128, F], f32, name="b3", tag="b3",
                                     bufs=B3_BUFS)
                        t2in = t2t.pop(t)
                        if t < T:
                            nc.tensor.matmul(b3[:, :], L3, t2in[:, :],
                                             start=True, stop=False)
                            g3 = t // 4
                            er = esb[:, 128 * (t % 2):128 * (t % 2) + 128]
                            c = ((t % 4) // 2) * F
                            nc.tensor.matmul(b3[:, :], er,
                                             dst[g3][:, c:c + F],
                                             start=False, stop=True)
                            if t % 4 == 3:
                                del dst[g3]
                        else:
                            nc.tensor.matmul(b3[:, :], L3, t2in[:, :],
                                             start=True, stop=True)
                        b3t[t] = b3

                    # ---- output DMA, after a group's last t3 write ----
                    if s >= 11 and (s - 11) % 4 == 0:
                        g = (s - 11) // 4
                        if g < TE // 4:
                            src = t3tile[g + SKEW]
                            if g < n_groups:
                                nc.gpsimd.dma_start(
                                    out=oT[:, g * 128:(g + 1) * 128],
                                    in_=src[64:72, :])
                            else:
                                gw = ((g - n_groups) % n_groups) * 128
                                nc.gpsimd.dma_start(
                                    out=oT[32:128, gw:gw + 128],
                                    in_=src[66:72, :])
                            if g >= 1:
                                t3tile.pop(g - 1, None)
                    # ---- ds load, one group ~4+ steps ahead ----
                    if s % 4 == 0:
                        gd = s // 4 + 1
                        if gd < n_groups:
                            d = io.tile([80, (G // 2) * F], bf16, name="ds",
                                        tag="ds", bufs=DS_BUFS)
                            nc.gpsimd.dma_start(
                                out=d[:],
                                in_=dT[:, gd * (G // 2) * F:
                                       (gd + 1) * (G // 2) * F])
                            dst[gd] = d
    nc.compile()
    return nc


def _host_prep(inputs):
    W_in, b_in = inputs["W_in"], inputs["b_in"]
    W0, b0 = inputs["W0"], inputs["b0"]
    Wd, bd = inputs["Wd"], inputs["bd"]
    Wc, bc = inputs["Wc"], inputs["bc"]
    W1a, b1a = inputs["W1a"], inputs["b1a"]
    W1b, b1b = inputs["W1b"], inputs["b1b"]
    Wo, bo = inputs["Wo"], inputs["bo"]

    Wc1 = (Wd[:, 1:].astype(np.float64) @ Wc[:15].astype(np.float64))
    bcp = (bd[1:].astype(np.float64) @ Wc[:15].astype(np.float64)
           + bc.astype(np.float64)).astype(np.float32)

    wblob = np.zeros((128, 384), np.float32)
    # L1: rows 0:64 = W1a (h3->h4pre) -> cols 0:64 ;
    #     rows 96:128 = W_in (X->h1pre) -> cols 64:128
    wblob[0:64, 0:64] = W1a
    wblob[96:128, 64:128] = W_in
    # L2: rows 0:64 = W1b (h4->h5pre) -> cols 64:128 ;
    #     rows 64:128 = W0 (h1->h2pre) -> cols 0:64
    wblob[0:64, 128 + 64:128 + 128] = W1b
    wblob[64:128, 128:128 + 64] = W0
    # L3: rows 0:64 (h2): Wc1 -> cols 0:64, +-Wd0 -> cols 64:66
    #     rows 64:128 (h5): +-Wo -> cols 66:72  (no biases: host adds)
    wblob[0:64, 256:256 + 64] = Wc1.astype(np.float32)
    wblob[0:64, 256 + 64] = Wd[:, 0]
    wblob[0:64, 256 + 65] = -Wd[:, 0]
    wblob[64:128, 256 + 66:256 + 69] = Wo
    wblob[64:128, 256 + 69:256 + 72] = -Wo

    # enc rider lhsT (bf16): rows 0:39 = Wc2, row 39 = h3 bias (const-1 rhs)
    # block 0 (cols 0:128) contracts rows 0:40 (even tiles),
    # block 1 (cols 128:256) contracts rows 40:80 (odd tiles)
    eblob = np.zeros((80, 256), np.float32)
    eblob[0:39, 0:64] = Wc[15:54]
    eblob[39, 0:64] = bcp
    eblob[40:79, 128:192] = Wc[15:54]
    eblob[79, 128:192] = bcp

    bblob = np.zeros((128, 2), np.float32)
    bblob[0:64, 0] = b1a
    bblob[64:128, 0] = b_in
    bblob[0:64, 1] = b0
    bblob[64:128, 1] = b1b

    np_bf = mybir.dt.np(bf16)
    emb = inputs["emb_points"]
    enc = inputs["enc_dir"]
    in_maps = []
    for cc in range(N_CORES):
        sl = slice(cc * NPC, (cc + 1) * NPC)
        encc = np.empty((40, NPC), np_bf)
        encc[0:39] = enc[sl].T.astype(np_bf)
        encc[39] = np.ones((NPC,), np_bf)
        # pair-interleave: [40, n_pairs, 2, F] -> rows 0:40 even tile,
        # rows 40:80 odd tile of each pair slot
        e4 = encc.reshape(40, NPC // (2 * F), 2, F)
        dpad = np.empty((80, NPC // 2), np_bf)
        dpad[0:40] = e4[:, :, 0, :].reshape(40, NPC // 2)
        dpad[40:80] = e4[:, :, 1, :].reshape(40, NPC // 2)
        in_maps.append({
            "xT": np.ascontiguousarray(emb[sl].T),
            "dT": dpad,
            "wb": wblob,
            "we": eblob.astype(np_bf),
            "bb": bblob,
        })
    return in_maps


_PROGRAM_CACHE = {}


def _get_program(npc=NPC, reps=1):
    key = (npc, reps)
    if key not in _PROGRAM_CACHE:
        _PROGRAM_CACHE[key] = build_program(npc, reps)
    return _PROGRAM_CACHE[key]


def kernel(**inputs) -> np.ndarray:
    nc = _get_program(NPC, 1)
    in_maps = _host_prep(inputs)
    res = run_bass_kernel_spmd(nc, in_maps, core_ids=list(range(N_CORES)))
    bd0 = float(inputs["bd"][0])
    bo = inputs["bo"].astype(np.float32)
    n_groups = NPC // (G * F)
    out = np.empty((N_TOTAL, 4), np.float32)
    for cc in range(N_CORES):
        op = res.results[cc]["oT"]         # [128, NPC//16] packed
        # unpack: col block g rows r <-> staging row 64+r//16,
        # col (r%16)*128+c2 of group g
        o = (op.reshape(128, n_groups, 128).transpose(1, 0, 2)
               .reshape(n_groups, 8, 16 * 128).transpose(1, 0, 2)
               .reshape(8, NPC))
        sl = slice(cc * NPC, (cc + 1) * NPC)
        out[sl, 3] = (o[0] - o[1]) + bd0   # dense (bias on host, exact)
        # color of tile t is stored at tile slot t+SKEW*G (mod n_tiles)
        col = (o[2:5] - o[5:8]) + bo[:, None]
        out[sl, 0:3] = np.roll(col, -SKEW * G * F, axis=1).T
    return out


# revision 38
# speedup vs baseline: 2.5764x; 2.1462x over previous
"""Trainium2 Bass kernel for the tiny NeRF MLP (nn_NeRFtinymodel).

Network (per point):
    h1 = relu(emb @ W_in + b_in)            # 32 -> 64
    h2 = relu(h1 @ W0 + b0)                 # 64 -> 64
    x3 = h2 @ Wd + bd                       # 64 -> 16 (no relu)
    dense = x3[:, 0]
    h3 = relu([x3[:,1:], enc_dir] @ Wc + bc)  # (15+39) -> 64
    h4 = relu(h3 @ W1a + b1a)
    h5 = relu(h4 @ W1b + b1b)
    color = h5 @ Wo + bo
    out = [color, dense]

Wd/Wc are algebraically fused on the host (no relu between them):
    h3 = relu(h2 @ (Wd[:,1:]@Wc[:15]) + enc_dir @ Wc[15:] + bc')
    dense = h2 @ Wd[:,0] + bd[0]

Device structure ("variant F"): activations transposed [features,
points], F=512 points per tile/chain.  Chain of tile t carries the
head of tile t and the tail of tile t-12 through three K=128/M=128
block-diagonal float32r matmuls plus a K=80 bf16 enc rider (39 enc
rows + a constant-1 row carrying the h3 bias, pair-interleaved so two
tiles share each rider rhs column block):

    P1(t): rhs=[h3(t-12) | X(t)]  -> [h4pre(t-12) | h1pre(t)]
    P2(t): rhs=[h4(t-12) | h1(t)] -> [h2pre(t)    | h5pre(t-12)]
    P3(t): rhs=[h2(t)    | h5(t-12)] -> [h3pre+bias(t) | den+-(t) | col+-(t-12)]
           + rider accumulating [enc(t);1] @ [Wc2;bc'] onto rows 0:64

Modulo software pipeline, one step per tile; PE stream at step s is
P1(s), P2(s-4), P3(s-6)+rider.  The cost model charges a DMA's full
per-partition-byte time (0.39 ns/B) to the ISSUING queue engine and
an eltwise op's column count to its engine, so the design splits work
four ways:  t1 as a [128,1024] PAIR op on DVE,  t2 per-tile on Act,
t3 per-tile alternating DVE/Act,  xT loads on SP,  ds/oT DMAs on the
otherwise-idle Pool queue (SWDGE).  PSUM: b1 pair (2 banks) x2 + b2
x2 + b3 x2 = 8 banks exactly.  The output is packed [128, npc/16] so
the store costs 500 ns instead of 3158 (host unpacks); den/color ride
relu(x)-relu(-x) with NO device bias; host subtracts and adds
bd[0]/bo (exact).

Sharding: pure data parallel over 8 cores on the points axis.
"""

import numpy as np

import concourse.bacc as bacc
import concourse.mybir as mybir
from concourse.tile import TileContext
from concourse.bass_utils import run_bass_kernel_spmd

N_CORES = 8
N_TOTAL = 1048576
NPC = N_TOTAL // N_CORES  # 131072 points per core
F = 512                   # points per tile (one PSUM bank)
G = 4                     # tiles per group (xT/oT DMA granularity)
SKEW = 3                  # cross-chain skew in groups (d = SKEW*G tiles)
B1_BUFS = 2               # [128, 2F] pairs
B2_BUFS = 2
B3_BUFS = 2
T1_BUFS = 2               # [128, 2F] pairs
T2_BUFS = 2
T3_BUFS = SKEW + 2
DS_BUFS = 4

f32 = mybir.dt.float32
f32r = mybir.dt.float32r
bf16 = mybir.dt.bfloat16
RELU = mybir.ActivationFunctionType.Relu
ADD = mybir.AluOpType.add
MAX = mybir.AluOpType.max


def build_program(npc=NPC, reps=1):
    assert npc % (G * F) == 0 and npc % 16 == 0
    n_groups = npc // (G * F)
    T = npc // F                 # live tiles/chains
    TE = T + SKEW * G            # incl. epilogue chains

    nc = bacc.Bacc("TRN2", target_bir_lowering=False, debug=False,
                   num_devices=N_CORES)
    xT = nc.dram_tensor("xT", [32, npc], f32r, kind="ExternalInput")
    dT = nc.dram_tensor("dT", [80, npc // 2], bf16, kind="ExternalInput")
    wb = nc.dram_tensor("wb", [128, 384], f32r, kind="ExternalInput")
    we = nc.dram_tensor("we", [80, 256], bf16, kind="ExternalInput")
    bb = nc.dram_tensor("bb", [128, 2], f32, kind="ExternalInput")
    # packed output: col block g holds the group-g [8, 2048] staging rows
    # in row-major 16-chunk order (host unpacks)
    oT = nc.dram_tensor("oT", [128, npc // 16], f32r, kind="ExternalOutput")

    with TileContext(nc) as tc:
        with (
            tc.tile_pool(name="wpool", bufs=1) as wpool,
            tc.tile_pool(name="io", bufs=2) as io,
            tc.tile_pool(name="act", bufs=3) as act,
            tc.tile_pool(name="ps", bufs=1, space="PSUM") as ps,
        ):
            wsb = wpool.tile([128, 384], f32r, name="wsb")
            esb = wpool.tile([80, 256], bf16, name="esb")
            bsb = wpool.tile([128, 2], f32, name="bsb")
            nc.sync.dma_start(out=wsb[:], in_=wb[:, :])
            nc.scalar.dma_start(out=esb[:], in_=we[:, :])
            nc.scalar.dma_start(out=bsb[:], in_=bb[:, :])
            L1 = wsb[:, 0:128]
            L2 = wsb[:, 128:256]
            L3 = wsb[:, 256:384]

            for rep in range(reps):
                t3tile = {}   # target-group -> [128, G*F] sbuf tile
                b1p, b2t, b3t = {}, {}, {}
                t1p, t2t, dst = {}, {}, {}
                t3_made = 0   # t3 tag instances created this rep

                # prime: zero stand-ins for chain-groups 0..SKEW-1.
                # Spread memsets/X loads over queues so the first chains
                # start as early as possible.
                x_eng = [nc.sync, nc.sync, nc.sync]
                m_eng = [nc.gpsimd, nc.gpsimd, nc.gpsimd]
                for g in range(SKEW):
                    tp = act.tile([128, G * F], f32r, name="t3p", tag="t3",
                                  bufs=T3_BUFS)
                    t3_made += 1
                    m_eng[g].memset(tp[0:96, :].bitcast(f32), 0.0)
                    # load X (any region for g >= n_groups: rows just need
                    # to be finite/owned; cheaper than a memset)
                    gx = min(g, n_groups - 1)
                    x_eng[g].dma_start(
                        out=tp[96:128, :],
                        in_=xT[:, gx * G * F:(gx + 1) * G * F])
                    t3tile[g] = tp
                if n_groups > 0:
                    d0 = io.tile([80, (G // 2) * F], bf16, name="ds",
                                 tag="ds", bufs=DS_BUFS)
                    nc.gpsimd.dma_start(out=d0[:], in_=dT[:, 0:(G // 2) * F])
                    dst[0] = d0

                for s in range(TE + 8):
                    # ---- eltwises for results of previous steps ----
                    u = s - 2
                    if 0 <= u < TE and u % 2 == 0:
                        # t1 pair q=u//2 = relu(b1p + bias0)  [DVE]
                        q = u // 2
                        t1 = act.tile([128, 2 * F], f32r, name="t1",
                                      tag="t1", bufs=T1_BUFS)
                        nc.vector.tensor_scalar(
                            out=t1[:], in0=b1p.pop(q)[:],
                            scalar1=bsb[:, 0:1], scalar2=0.0,
                            op0=ADD, op1=MAX)
                        t1p[q] = t1
                    u = s - 5
                    if 0 <= u < TE:      # t2 = relu(b2 + bias1)  [Act]
                        t2 = act.tile([128, F], f32r, name="t2", tag="t2",
                                      bufs=T2_BUFS)
                        nc.scalar.activation(t2[:], b2t.pop(u)[:], RELU,
                                             bias=bsb[:, 1:2], scale=1.0)
                        t2t[u] = t2
                    u = s - 7
                    if 0 <= u < TE:
                        # t3(u): per-tile relu of rows 0:96 [DVE/Act alt]
                        gt = u // 4 + SKEW
                        if gt not in t3tile:
                            t3tile[gt] = act.tile(
                                [128, G * F], f32r, name="t3w",
                                tag="t3", bufs=T3_BUFS)
                            t3_made += 1
                            # real X, or (epilogue) any X region: rows
                            # 96:128 just need to be owned/finite for P1
                            gx = min(gt, n_groups - 1)
                            nc.sync.dma_start(
                                out=t3tile[gt][96:128, :],
                                in_=xT[:, gx * G * F:
                                       (gx + 1) * G * F])
                        c0 = (u % 4) * F
                        dstv = t3tile[gt][0:96, c0:c0 + F]
                        srcv = b3t.pop(u)[0:96, :]
                        if u % 2 == 1:
                            nc.vector.tensor_scalar(
                                out=dstv, in0=srcv, scalar1=0.0,
                                scalar2=None, op0=MAX)
                        else:
                            nc.scalar.activation(dstv, srcv, RELU,
                                                 bias=0.0, scale=1.0)

                    # ---- matmuls ----
                    if s < TE:           # P1(s) into b1 pair slot
                        t = s
                        q = t // 2
                        if t % 2 == 0:
                            b1p[q] = ps.tile([128, 2 * F], f32, name="b1",
                                             tag="b1", bufs=B1_BUFS)
                        rhs = t3tile[t // 4][:, (t % 4) * F:(t % 4 + 1) * F]
                        nc.tensor.matmul(
                            b1p[q][:, (t % 2) * F:(t % 2 + 1) * F],
                            L1, rhs, start=True, stop=True)
                    t = s - 4
                    if 0 <= t < TE:      # P2(t)
                        b2 = ps.tile([128, F], f32, name="b2", tag="b2",
                                     bufs=B2_BUFS)
                        nc.tensor.matmul(
                            b2[:, :], L2,
                            t1p[t // 2][:, (t % 2) * F:(t % 2 + 1) * F],
                            start=True, stop=True)
                        b2t[t] = b2
                        if t % 2 == 1:
                            del t1p[t // 2]
                    t = s - 6
                    if 0 <= t < TE:      # P3(t) (+ rider for live tiles)
                        b3 = ps.tile([128, F], f32, name="b3", tag="b3",
                                     bufs=B3_BUFS)
                        t2in = t2t.pop(t)
                        if t < T:
                            nc.tensor.matmul(b3[:, :], L3, t2in[:, :],
                                             start=True, stop=False)
                            g3 = t // 4
                            er = esb[:, 128 * (t % 2):128 * (t % 2) + 128]
                            c = ((t % 4) // 2) * F
                            nc.tensor.matmul(b3[:, :], er,
                                             dst[g3][:, c:c + F],
                                             start=False, stop=True)
                            if t % 4 == 3:
                                del dst[g3]
                        else:
                            nc.tensor.matmul(b3[:, :], L3, t2in[:, :],
                                             start=True, stop=True)
                        b3t[t] = b3

                    # ---- output DMA, after a group's last t3 write ----
                    if s >= 11 and (s - 11) % 4 == 0:
                        g = (s - 11) // 4
                        if g < TE // 4:
                            src = t3tile[g + SKEW]
                            if g < n_groups:
                                nc.gpsimd.dma_start(
                                    out=oT[:, g * 128:(g + 1) * 128],
                                    in_=src[64:72, :])
                            else:
                                gw = ((g - n_groups) % n_groups) * 128
                                nc.gpsimd.dma_start(
                                    out=oT[32:128, gw:gw + 128],
                                    in_=src[66:72, :])
                            if g >= 1:
                                t3tile.pop(g - 1, None)
                    # ---- ds load, one group ~4+ steps ahead ----
                    if s % 4 == 0:
                        gd = s // 4 + 1
                        if gd < n_groups:
                            d = io.tile([80, (G // 2) * F], bf16, name="ds",
                                        tag="ds", bufs=DS_BUFS)
                            nc.gpsimd.dma_start(
                                out=d[:],
                                in_=dT[:, gd * (G // 2) * F:
                                       (gd + 1) * (G // 2) * F])
                            dst[gd] = d
    nc.compile()
    return nc


def _host_prep(inputs):
    W_in, b_in = inputs["W_in"], inputs["b_in"]
    W0, b0 = inputs["W0"], inputs["b0"]
    Wd, bd = inputs["Wd"], inputs["bd"]
    Wc, bc = inputs["Wc"], inputs["bc"]
    W1a, b1a = inputs["W1a"], inputs["b1a"]
    W1b, b1b = inputs["W1b"], inputs["b1b"]
    Wo, bo = inputs["Wo"], inputs["bo"]

    Wc1 = (Wd[:, 1:].astype(np.float64) @ Wc[:15].astype(np.float64))
    bcp = (bd[1:].astype(np.float64) @ Wc[:15].astype(np.float64)
           + bc.astype(np.float64)).astype(np.float32)

    wblob = np.zeros((128, 384), np.float32)
    # L1: rows 0:64 = W1a (h3->h4pre) -> cols 0:64 ;
    #     rows 96:128 = W_in (X->h1pre) -> cols 64:128
    wblob[0:64, 0:64] = W1a
    wblob[96:128, 64:128] = W_in
    # L2: rows 0:64 = W1b (h4->h5pre) -> cols 64:128 ;
    #     rows 64:128 = W0 (h1->h2pre) -> cols 0:64
    wblob[0:64, 128 + 64:128 + 128] = W1b
    wblob[64:128, 128:128 + 64] = W0
    # L3: rows 0:64 (h2): Wc1 -> cols 0:64, +-Wd0 -> cols 64:66
    #     rows 64:128 (h5): +-Wo -> cols 66:72  (no biases: host adds)
    wblob[0:64, 256:256 + 64] = Wc1.astype(np.float32)
    wblob[0:64, 256 + 64] = Wd[:, 0]
    wblob[0:64, 256 + 65] = -Wd[:, 0]
    wblob[64:128, 256 + 66:256 + 69] = Wo
    wblob[64:128, 256 + 69:256 + 72] = -Wo

    # enc rider lhsT (bf16): rows 0:39 = Wc2, row 39 = h3 bias (const-1 rhs)
    # block 0 (cols 0:128) contracts rows 0:40 (even tiles),
    # block 1 (cols 128:256) contracts rows 40:80 (odd tiles)
    eblob = np.zeros((80, 256), np.float32)
    eblob[0:39, 0:64] = Wc[15:54]
    eblob[39, 0:64] = bcp
    eblob[40:79, 128:192] = Wc[15:54]
    eblob[79, 128:192] = bcp

    bblob = np.zeros((128, 2), np.float32)
    bblob[0:64, 0] = b1a
    bblob[64:128, 0] = b_in
    bblob[0:64, 1] = b0
    bblob[64:128, 1] = b1b

    np_bf = mybir.dt.np(bf16)
    emb = inputs["emb_points"]
    enc = inputs["enc_dir"]
    in_maps = []
    for cc in range(N_CORES):
        sl = slice(cc * NPC, (cc + 1) * NPC)
        encc = np.empty((40, NPC), np_bf)
        encc[0:39] = enc[sl].T.astype(np_bf)
        encc[39] = np.ones((NPC,), np_bf)
        # pair-interleave: [40, n_pairs, 2, F] -> rows 0:40 even tile,
        # rows 40:80 odd tile of each pair slot
        e4 = encc.reshape(40, NPC // (2 * F), 2, F)
        dpad = np.empty((80, NPC // 2), np_bf)
        dpad[0:40] = e4[:, :, 0, :].reshape(40, NPC // 2)
        dpad[40:80] = e4[:, :, 1, :].reshape(40, NPC // 2)
        in_maps.append({
            "xT": np.ascontiguousarray(emb[sl].T),
            "dT": dpad,
            "wb": wblob,
            "we": eblob.astype(np_bf),
            "bb": bblob,
        })
    return in_maps


_PROGRAM_CACHE = {}


def _get_program(npc=NPC, reps=1):
    key = (npc, reps)
    if key not in _PROGRAM_CACHE:
        _PROGRAM_CACHE[key] = build_program(npc, reps)
    return _PROGRAM_CACHE[key]


def kernel(**inputs) -> np.ndarray:
    nc = _get_program(NPC, 1)
    in_maps = _host_prep(inputs)
    res = run_bass_kernel_spmd(nc, in_maps, core_ids=list(range(N_CORES)))
    bd0 = float(inputs["bd"][0])
    bo = inputs["bo"].astype(np.float32)
    n_groups = NPC // (G * F)
    out = np.empty((N_TOTAL, 4), np.float32)
    for cc in range(N_CORES):
        op = res.results[cc]["oT"]         # [128, NPC//16] packed
        # unpack: col block g rows r <-> staging row 64+r//16,
        # col (r%16)*128+c2 of group g
        o = (op.reshape(128, n_groups, 128).transpose(1, 0, 2)
               .reshape(n_groups, 8, 16 * 128).transpose(1, 0, 2)
               .reshape(8, NPC))
        sl = slice(cc * NPC, (cc + 1) * NPC)
        out[sl, 3] = (o[0] - o[1]) + bd0   # dense (bias on host, exact)
        # color of tile t is stored at tile slot t+SKEW*G (mod n_tiles)
        col = (o[2:5] - o[5:8]) + bo[:, None]
        out[sl, 0:3] = np.roll(col, -SKEW * G * F, axis=1).T
    return out
